# revision 35
# baseline (speedup 1.0000x reference)
"""Trainium2 Bass kernel for MultiHeadGlobalAttention2d.

Sharding (8 cores): core = (batch b, head-group g), b in 0..3, g in 0..1.
Each core computes, for its batch and its 4 heads (128 channels):
  q/k/v projections, attention (softmax over keys), and the partial output
  projection  y_part = Wo[:, ch_slice] @ att_out.
Host sums the two partials per batch and adds the output bias.

Design (exp throughput is the limiting resource; spread it over two engines
and keep both saturated):
  - S^T orientation: score tiles [keys(m) on partitions, queries(n) on free].
    Exact exp on ScalarE straight out of PSUM; ~40% of score tiles instead
    use a Schraudolph fast-exp on VectorE (one MULT+ADD with int16 output
    bitcast as bf16), splitting the exp stream across both engines.
  - QK matmuls 4-head row-tiled (tile_position), concurrent on the PE array.
  - Fused AV+den: stationary [vT_h | ones] (64 cols) yields [AV_h; den_h]
    in ONE matmul per head; the two heads of an es-pair go to different
    PSUM banks AND different array col-groups so they run concurrently.
  - vT built directly by PE matmul (x_v^T stationary, Wv^T moving) into an
    interleaved [vT_h | ones] layout; no PE transposes.
  - All projections f16 (full rate + FWL); x inputs DMA'd in 512-col chunks
    across many queues; k/q/v projection work packed into single borrowed
    PSUM slots interleaved with the first attention block.
  - AV/den matmuls run 6 iterations behind the QK/exp stream (es tiles
    buffer the lag) so neither exp engine ever waits on the accumulators.
  - Per-block epilogue: full-bank reciprocal_approx_fast, cross-quadrant
    32-part muls (PSUM/SBUF base mismatch), out-projection with zero-padded
    permuted Wo halves; emitted inside the next block so the exp stream
    never stalls at block boundaries.
"""

import numpy as np

B = 4
CIN = 256
COUT = 256
HH = 48
WW = 48
N = HH * WW            # 2304
D = 32                 # head dim
NHL = 4                # heads per core
HGC = NHL * D          # 128 channels per head-group
NCORES = 8
NBLK = [(0, 512), (512, 512), (1024, 512), (1536, 512), (2048, 256)]
NMT = N // 128         # 18 key tiles
XCH = [(0, 512), (512, 512), (1024, 512), (1536, 512), (2048, 256)]  # DMA chunks

_PROG = {}


def build_program():
    if "nc" in _PROG:
        return _PROG["nc"]

    from contextlib import ExitStack

    import concourse.bacc as bacc
    import concourse.mybir as mybir
    import concourse.tile as tile

    f32 = mybir.dt.float32
    bf16 = mybir.dt.bfloat16
    f16 = mybir.dt.float16
    i16 = mybir.dt.int16
    EXP = mybir.ActivationFunctionType.Exp
    MULT = mybir.AluOpType.mult
    ADD = mybir.AluOpType.add
    # DVE fast-exp: bitcast(int16(s*(128*log2e)/16 + (127*128 - C))) read as
    # bf16 approximates exp(s/16) (Schraudolph); C tuned for min max-rel-err.
    FE_A = 128.0 * 1.4426950408889634 / 16.0
    FE_B = 127.0 * 128.0 - 5.5

    nc = bacc.Bacc("TRN2", target_bir_lowering=False, debug=False)

    xq_d = nc.declare_dram_parameter("xq", [CIN, N], f16, False)
    xk_d = nc.declare_dram_parameter("xk", [CIN, N], f16, False)
    xv_d = nc.declare_dram_parameter("xv", [CIN, N], f16, False)
    wqT_d = nc.declare_dram_parameter("wqT", [CIN, HGC], f16, False)
    wkT_d = nc.declare_dram_parameter("wkT", [CIN, HGC], f16, False)
    wvT_d = nc.declare_dram_parameter("wvT", [CIN, HGC], f16, False)
    woA_d = nc.declare_dram_parameter("woA", [128, COUT], bf16, False)
    woB_d = nc.declare_dram_parameter("woB", [128, COUT], bf16, False)
    bq_d = nc.declare_dram_parameter("bq", [HGC, 1], f32, False)
    bk_d = nc.declare_dram_parameter("bk", [HGC, 1], f32, False)
    bvrep_d = nc.declare_dram_parameter("bvrep", [128, HGC], f16, False)
    y_d = nc.declare_dram_parameter("y", [COUT, N], f32, True)

    with tile.TileContext(nc) as tc, ExitStack() as ctx:
        const = ctx.enter_context(tc.tile_pool(name="const", bufs=1))
        resid = ctx.enter_context(tc.tile_pool(name="resid", bufs=1))
        xin = ctx.enter_context(tc.tile_pool(name="xin", bufs=1))
        espool = ctx.enter_context(tc.tile_pool(name="espool", bufs=16))
        trans = ctx.enter_context(tc.tile_pool(name="trans", bufs=2))
        ps_s = ctx.enter_context(tc.tile_pool(name="ps_s", bufs=3, space="PSUM"))
        ps_a = ctx.enter_context(tc.tile_pool(name="ps_a", bufs=2, space="PSUM"))

        # ---- constants / weights (small, issued first) ----
        wq = const.tile([128, 2, 128], f16)
        wk = const.tile([128, 2, 128], f16)
        wv = const.tile([128, 2, 128], f16)
        for wt, wd in ((wk, wkT_d), (wq, wqT_d), (wv, wvT_d)):
            nc.sync.dma_start(wt[:, :, :], wd[:, :].rearrange("(c p) k -> p c k", p=128))
        woA = const.tile([128, COUT], bf16)
        woB = const.tile([128, COUT], bf16)
        nc.sync.dma_start(woA[:, :], woA_d[:, :])
        nc.sync.dma_start(woB[:, :], woB_d[:, :])
        bq_s = const.tile([128, 1], f32)
        bk_s = const.tile([128, 1], f32)
        for bt, bd in ((bk_s, bk_d), (bq_s, bq_d)):
            nc.sync.dma_start(bt[:, :], bd[:, :])
        bvrep = const.tile([128, 128], f16)
        nc.sync.dma_start(bvrep[:, :], bvrep_d[:, :])

        # ---- x inputs: 512-col chunks, k/q/v interleaved so the front of
        # each tensor lands early and chunks spread across DMA queues ----
        xk_t = xin.tile([128, 2, N], f16)
        xq_t = xin.tile([128, 2, N], f16)
        xv_t = xin.tile([128, 2, N], f16)
        def xdma(xt, xd, lo, sz):
            nc.sync.dma_start(
                xt[:, :, lo : lo + sz],
                xd[:, lo : lo + sz].rearrange("(c p) n -> p c n", p=128),
            )
        # 256-col leading pieces so the first projections start sooner
        for xt, xd, lo, sz in (
            (xk_t, xk_d, 0, 256), (xq_t, xq_d, 0, 256),
            (xk_t, xk_d, 256, 256), (xq_t, xq_d, 256, 256),
            (xk_t, xk_d, 512, 512), (xv_t, xv_d, 0, 512),
            (xk_t, xk_d, 1024, 512), (xv_t, xv_d, 512, 512),
            (xk_t, xk_d, 1536, 512), (xk_t, xk_d, 2048, 256),
            (xv_t, xv_d, 1024, 512), (xq_t, xq_d, 512, 512),
            (xv_t, xv_d, 1536, 512), (xv_t, xv_d, 2048, 256),
            (xq_t, xq_d, 1024, 512), (xq_t, xq_d, 1536, 512),
            (xq_t, xq_d, 2048, 256),
        ):
            xdma(xt, xd, lo, sz)

        # ---- residents ----
        q_sb = resid.tile([128, N], f16)
        k_sb = resid.tile([128, N], f16)
        vT2_sb = resid.tile([128, 2 * N], bf16)
        recTA = resid.tile([128, 512], f32)
        recTB = resid.tile([128, 512], f32)
        attA_sb = resid.tile([128, 512], bf16)
        attB_sb = resid.tile([128, 512], bf16)
        # ones strips (cols 32:64 of each 64-col head group), written once;
        # att junk lanes (den positions) stay zero forever
        nc.vector.memset(
            vT2_sb[:, :].rearrange("p (g t) -> p g t", t=64)[:, :, D:64], 1.0)
        nc.vector.memset(attA_sb[:, :], 0.0)
        nc.vector.memset(attB_sb[:, :], 0.0)

        # PE prologue absorb for const tiles consumed by PE (x chunks carry
        # their DMA wait on the consuming matmul directly).
        def absorb(tiles):
            scr = ps_s.tile([128, 512], f32, tag="s", name="scr")
            for t in tiles:
                tv = t[:, :, :].rearrange("p c k -> p (c k)") if t.ndim == 3 else t[:, :]
                F = tv.shape[-1]
                M = min(F, 128)
                W = min(F, 2)
                nc.tensor.matmul(
                    scr[0:M, 0:W], tv[0:1, F - M : F], tv[0:1, F - W : F],
                    start=True, stop=True,
                )

        def proj_piece(pp, po_, w, xt, brow, dst, o, sz):
            """dst[:, o:o+sz] = (W_h @ x)[:, o:o+sz] + b ; pp[:, po_:po_+sz] scratch."""
            nc.tensor.matmul(
                pp[:, po_ : po_ + sz], w[:, 0, :], xt[:, 0, o : o + sz],
                start=True, stop=False,
            )
            nc.tensor.matmul(
                pp[:, po_ : po_ + sz], w[:, 1, :], xt[:, 1, o : o + sz],
                start=False, stop=True,
            )
            nc.vector.tensor_scalar_add(dst[:, o : o + sz], pp[:, po_ : po_ + sz], brow[:, 0:1])

        def vt_piece(pt, po_, j):
            """vT_sb[:, 128j:+128] = (x_v[:, 128j:+128])^T @ Wv^T + bv."""
            nc.tensor.matmul(
                pt[:, po_ : po_ + 128], xv_t[:, 0, 128 * j : 128 * j + 128], wv[:, 0, :],
                start=True, stop=False,
            )
            nc.tensor.matmul(
                pt[:, po_ : po_ + 128], xv_t[:, 1, 128 * j : 128 * j + 128], wv[:, 1, :],
                start=False, stop=True,
            )
            vstrip = vT2_sb[:, 256 * j : 256 * j + 256].rearrange(
                "p (h t) -> p h t", h=NHL)[:, :, 0:D]
            nc.vector.tensor_add(
                vstrip,
                pt[:, po_ : po_ + 128].rearrange("p (h t) -> p h t", h=NHL),
                bvrep[:, :].rearrange("p (h t) -> p h t", h=NHL),
            )

        def work_group(items):
            # pack several proj/vT pieces into ONE borrowed PSUM slot so the
            # score-slot rotation loses at most one slot per group
            pp = ps_s.tile([128, 1024], f32, tag="s", name="wg")
            cur = 0
            for it in items:
                if it[0] == "proj":
                    _, w, xt, brow, dst, o, sz = it
                    proj_piece(pp, cur, w, xt, brow, dst, o, sz)
                    cur += sz
                else:
                    _, j = it
                    vt_piece(pp, cur, j)
                    cur += 128
            assert cur <= 1024

        # ---- prologue: minimum to start the exp stream ----
        absorb([wk, wq, bk_s, bq_s])
        work_group([("proj", wk, xk_t, bk_s, k_sb, 0, 512)])
        work_group([("proj", wq, xq_t, bq_s, q_sb, 0, 512)])
        absorb([wv, bvrep, woA, woB])

        # epilogue state carried across blocks
        pending = []

        def emit_norm_half(o, sz, acc, rec, att):
            # acc layout per bank: [AV_e(32p); den_e(32p); AV_o(32p); den_o(32p)].
            # Full-bank aligned recip, then 32-part muls with the SBUF operand
            # cross-quadrant (PSUM+SBUF base mismatch is legal): att valid
            # lanes = AV*1/den; junk lanes stay memset-0.
            nc.vector.reciprocal_approx_fast(rec[:, :sz], acc[:, :sz])
            nc.vector.tensor_mul(att[0:32, :sz], acc[0:32, :sz], rec[32:64, :sz])
            nc.vector.tensor_mul(att[64:96, :sz], acc[64:96, :sz], rec[96:128, :sz])

        def emit_po(o, sz):
            attA, attB = attA_sb, attB_sb
            for cc in range(2):
                po = ps_a.tile([128, 512], f32, tag="acc", name="po")
                pv = po[:, :sz]
                nc.tensor.matmul(
                    pv, woA[:, 128 * cc : 128 * cc + 128], attA[:, :sz],
                    start=True, stop=False,
                )
                nc.tensor.matmul(
                    pv, woB[:, 128 * cc : 128 * cc + 128], attB[:, :sz],
                    start=False, stop=True,
                )
                yt = trans.tile([128, 512], f32, tag="yt")
                nc.vector.tensor_copy(yt[:, :sz], pv)
                nc.sync.dma_start(y_d[128 * cc : 128 * cc + 128, o : o + sz], yt[:, :sz])

        # interleave schedule: (block, j) -> one packed borrow each
        interleave = {
            (0, 0): lambda: work_group([("proj", wk, xk_t, bk_s, k_sb, 512, 512),
                                        ("vt", 4), ("vt", 5)]),
            (0, 1): lambda: work_group([("proj", wk, xk_t, bk_s, k_sb, 1024, 512),
                                        ("vt", 6), ("vt", 7)]),
            (0, 2): lambda: work_group([("proj", wk, xk_t, bk_s, k_sb, 1536, 512),
                                        ("vt", 8), ("vt", 9)]),
            (0, 3): lambda: work_group([("proj", wk, xk_t, bk_s, k_sb, 2048, 256),
                                        ("vt", 10), ("vt", 11), ("vt", 12)]),
            (0, 4): lambda: work_group([("vt", 13), ("vt", 14), ("vt", 15),
                                        ("vt", 16), ("vt", 0), ("vt", 1)]),
            (0, 5): lambda: work_group([("proj", wq, xq_t, bq_s, q_sb, 512, 512),
                                        ("vt", 17), ("vt", 2), ("vt", 3)]),
            (0, 8): lambda: work_group([("proj", wq, xq_t, bq_s, q_sb, 1024, 512)]),
            (1, 5): lambda: work_group([("proj", wq, xq_t, bq_s, q_sb, 1536, 512)]),
            (2, 5): lambda: work_group([("proj", wq, xq_t, bq_s, q_sb, 2048, 256)]),
        }

        # ---- attention ----
        for bi, (o, sz) in enumerate(NBLK):
            pend_av = []
            accbox = []

            def emit_av(item, accbox=accbox, sz=sz):
                if not accbox:
                    accbox.append(ps_a.tile([128, 512], f32, tag="acc", name="accA"))
                    accbox.append(ps_a.tile([128, 512], f32, tag="acc", name="accB"))
                accA, accB = accbox
                j, ess = item
                # fused AV+den: stationary [vT_h | ones] (64 cols) produces
                # [AV_h (32p) ; den_h (32p)] per head in one matmul
                # per es-pair the two heads go to different banks AND
                # different array col-groups so they run concurrently:
                # h0 -> accA@0, h1 -> accB@64, h2 -> accA@64, h3 -> accB@0
                PAH = {0: (0, 0), 1: (1, 64), 2: (0, 64), 3: (1, 0)}
                for hp in range(2):
                    es = ess[hp]
                    for hh in range(2):
                        h = 2 * hp + hh
                        bk_i, pa = PAH[h]
                        acc = accA if bk_i == 0 else accB
                        nc.tensor.matmul(
                            acc[pa : pa + 64, :sz],
                            vT2_sb[:, 256 * j + 64 * h : 256 * j + 64 * h + 64],
                            es[:, 512 * hh : 512 * hh + sz],
                            start=(j == 0), stop=(j == NMT - 1),
                            tile_position=(0, pa),
                        )

            for j in range(NMT):
                s2s = []
                for hp in range(2):  # head pair: heads (2*hp, 2*hp+1)
                    s2 = ps_s.tile([128, 1024], f32, tag="s")
                    s2s.append(s2)
                    for hh in range(2):
                        h = 2 * hp + hh
                        nc.tensor.matmul(
                            s2[:, 512 * hh : 512 * hh + sz],
                            k_sb[32 * h : 32 * h + 32, 128 * j : 128 * j + 128],
                            q_sb[32 * h : 32 * h + 32, o : o + sz],
                            start=True, stop=True, tile_position=(32 * h, 0),
                        )
                ess = []
                for hp in range(2):
                    s2 = s2s[hp]
                    es = espool.tile([128, 1024], bf16, tag="es")
                    ess.append(es)
                    # hp1 tiles go to DVE fast-exp on 5 of every 6 j's
                    on_dve = (hp == 1) and (j % 6 != 5)
                    if sz == 512:
                        if on_dve:
                            nc.vector.tensor_scalar(
                                es[:, :].bitcast(i16), s2[:, :], FE_A, FE_B, MULT, ADD)
                        else:
                            nc.scalar.activation(es[:, :], s2[:, :], EXP, scale=1.0 / 16.0)
                    else:
                        sv = s2[:, :].rearrange("p (b x) -> p b x", b=2)[:, :, :sz]
                        ev = es[:, :].rearrange("p (b x) -> p b x", b=2)[:, :, :sz]
                        if on_dve:
                            nc.vector.tensor_scalar(ev.bitcast(i16), sv, FE_A, FE_B, MULT, ADD)
                        else:
                            nc.scalar.activation(ev, sv, EXP, scale=1.0 / 16.0)
                pend_av.append((j, ess))
                if len(pend_av) > 2 and j >= 6:
                    emit_av(pend_av.pop(0))
                # drain the backlog before every block end so the next
                # block's QK/exp stream isn't stuck behind an AV burst
                if j >= 12 and len(pend_av) > 1:
                    emit_av(pend_av.pop(0))
                # deferred epilogue of the previous block: DVE normalization
                # at j==1, PE out-projection at j==3
                if j == 0 and pending:
                    po_, psz_, paccA, paccB = pending[0]
                    emit_norm_half(po_, psz_, paccA, recTA, attA_sb)
                if j == 1 and pending:
                    po_, psz_, paccA, paccB = pending[0]
                    emit_norm_half(po_, psz_, paccB, recTB, attB_sb)
                if j == 2 and pending:
                    po_args = pending.pop(0)
                    emit_po(po_args[0], po_args[1])
                # interleave: finish projections + vT chunks
                if (bi, j) in interleave:
                    interleave[(bi, j)]()
            while pend_av:
                emit_av(pend_av.pop(0))
            pending.append((o, sz, accbox[0], accbox[1]))
        fo_, fsz_, faccA, faccB = pending[0]
        emit_norm_half(fo_, fsz_, faccA, recTA, attA_sb)
        emit_norm_half(fo_, fsz_, faccB, recTB, attB_sb)
        po_args = pending.pop(0)
        emit_po(po_args[0], po_args[1])

    nc.compile()

    _PROG["nc"] = nc
    return nc


def make_in_maps(inputs):
    """Shard full inputs into the 8 per-core input maps."""
    import ml_dtypes

    bf16 = ml_dtypes.bfloat16
    g = {k: np.asarray(v) for k, v in inputs.items()}
    xq_b = [np.ascontiguousarray(g["queries"][b].reshape(CIN, N).astype(np.float16)) for b in range(B)]
    xk_b = [np.ascontiguousarray(g["keys"][b].reshape(CIN, N).astype(np.float16)) for b in range(B)]
    xv_b = [np.ascontiguousarray(g["values"][b].reshape(CIN, N).astype(np.float16)) for b in range(B)]
    def _wo_perm(woT, half):
        # bank A holds att rows [h0 @0:32, h2 @64:96]; bank B [h3 @0:32, h1 @64:96]
        out = np.zeros((128, COUT), dtype=woT.dtype)
        if half == 0:
            out[0:32] = woT[0:32]
            out[64:96] = woT[64:96]
        else:
            out[0:32] = woT[96:128]
            out[64:96] = woT[32:64]
        return out
    in_maps = []
    for core in range(NCORES):
        b, grp = divmod(core, 2)
        hs = slice(grp * HGC, (grp + 1) * HGC)
        in_maps.append({
            "xq": xq_b[b],
            "xk": xk_b[b],
            "xv": xv_b[b],
            "wqT": np.ascontiguousarray(g["Wq"][hs, :].T.astype(np.float16)),
            "wkT": np.ascontiguousarray(g["Wk"][hs, :].T.astype(np.float16)),
            "wvT": np.ascontiguousarray(g["Wv"][hs, :].T.astype(np.float16)),
            "woA": _wo_perm(g["Wo"][:, hs].T.astype(bf16), 0),
            "woB": _wo_perm(g["Wo"][:, hs].T.astype(bf16), 1),
            "bq": np.ascontiguousarray(g["bq"][hs].reshape(HGC, 1).astype(np.float32)),
            "bk": np.ascontiguousarray(g["bk"][hs].reshape(HGC, 1).astype(np.float32)),
            "bvrep": np.ascontiguousarray(
                np.broadcast_to(g["bv"][hs].reshape(1, HGC), (128, HGC)).astype(np.float16)),
        })
    return in_maps


def unshard(results, bo):
    parts = [results[i]["y"] for i in range(NCORES)]
    out = np.empty((B, COUT, N), dtype=np.float32)
    for b in range(B):
        out[b] = parts[2 * b] + parts[2 * b + 1]
    out += np.asarray(bo, dtype=np.float32).reshape(1, COUT, 1)
    return out.reshape(B, COUT, HH, WW)


def kernel(**inputs):
    from concourse.bass_utils import run_bass_kernel_spmd

    nc = build_program()
    in_maps = make_in_maps(inputs)
    res = run_bass_kernel_spmd(nc, in_maps, list(range(NCORES)))
    return unshard(res.results, inputs["bo"])
